# revision 15
# baseline (speedup 1.0000x reference)
"""Trainium2 Bass kernel for nn_BaseModel_74302934220896 (TuckER + possibility-codebook).

Contract: kernel(**inputs) takes FULL unsharded inputs (as in reference.setup_inputs())
and returns the full output tuple (tucker_logits [B,N] f32, possibility_score [B,N] f32).

Sharding (8 cores):
  - B (2048) -> 8 x 256 for relation / hr / Wm / codebook paths
  - N (20000) -> 8 x 2500 (padded to 2560) for tail features and the [B,N] score matmuls
  - head MLP replicated over full B on every core so BN0 needs no collective
  - codebook is tanh'ed and gathered per-row on the HOST (pcg input)
  - two small bf16 AllGathers (WmT first, interT second); BN1 stats computed locally
    from the gathered full-B WmT.

v4 notes: all weights/activations are host-packed into a handful of big DMA blobs
(one HWDGE trigger each, ~620ns per trigger on the sequencer), loads split across
the SP and ACT DGE rings, AllGather path uses DMA-xbar transposes (off the PE
queue), masks deferred past BN1, score evacuations split DVE/ACT, outputs staged
[128, 2560] bf16 with 5KB-line DMAs.
"""

import sys

sys.path.insert(0, "/opt/trn_rl_repo")

import numpy as np
import ml_dtypes

import concourse.bass as bass
import concourse.bacc as bacc
import concourse.mybir as mybir
import concourse.tile as tile
from concourse.bass_utils import run_bass_kernel_spmd
from concourse.masks import make_identity

F32 = mybir.dt.float32
BF16 = mybir.dt.bfloat16
AF = mybir.ActivationFunctionType
ALU = mybir.AluOpType
AX = mybir.AxisListType

B, N, E, C, R2 = 2048, 20000, 512, 128, 474
NCORES = 8
BSH = B // NCORES            # 256 b rows per core (sharded paths)
NSH = N // NCORES            # 2500 tail rows per core
NPAD = 2560                  # padded to 5 groups of 512
NG = NPAD // 512             # 5 n-groups
NB_FULL = B // 128           # 16 b-tiles over full B
TEMP = 0.5
NEG = -1.0e30
HALF = C * C // 2

# const blob column layout (f32 [128, 27])
_CB = dict(hsb1=0, rsb1=4, tsb1=8, tab1=12, hrb1=16, hrb2=18, hrb3=20, rsb2=21,
           tsb2=22, bn0g=23, bn0b=24, bn1g=25, bn1b=26)
# w2 blob column layout (bf16 [128, 2816]); entries are (offset, m, cols)
_W2 = dict(hsw2=(0, 4, 128), rsw2=(512, 4, 128), tsw2=(1024, 4, 128),
           taw2=(1536, 4, 128), hrw3=(2048, 2, 128), hrw2=(2304, 2, 256))
# w1 blob column layout (bf16 [128, 10240]); entries are (offset, nk, cols)
_W1 = dict(hsw1=(0, 4, 512), rsw1=(2048, 4, 512), tsw1=(4096, 4, 512),
           taw1=(6144, 4, 512), hrw1=(8192, 8, 256))

_PROG_CACHE = {}


def build_program():
    nc = bacc.Bacc("TRN2", target_bir_lowering=False, debug=False,
                   num_devices=NCORES)

    # ---------------- DRAM I/O ----------------
    dI = lambda name, shape, dt=BF16: nc.dram_tensor(name, shape, dt, kind="ExternalInput")
    headT = dI("headT", [128, 4, B])                   # full-B head^T, [p, k, b]
    relT = dI("relT", [128, 4, BSH])
    tailT = dI("tailT", [128, 4, NPAD])
    pcg = dI("pcg", [2, 2, 128, HALF])                 # host-gathered tanh codebook
    core2 = dI("core2", [C, C * C])                    # core reshaped [e, (c,d)]
    w1b = dI("w1b", [128, 10240])
    w2b = dI("w2b", [128, 2816])
    cb = dI("cb", [128, 27], F32)
    tab2r = dI("tab2r", [1, C])

    tucker = nc.dram_tensor("tucker", [B, NSH], BF16, kind="ExternalOutput")
    poss = nc.dram_tensor("poss", [B, NSH], BF16, kind="ExternalOutput")

    with tile.TileContext(nc) as tc:
        with (
            tc.tile_pool(name="const", bufs=1) as constp,
            tc.tile_pool(name="big", bufs=2) as bigp,
            tc.tile_pool(name="h1", bufs=8) as h1p,
            tc.tile_pool(name="pers", bufs=1) as pers,
            tc.tile_pool(name="small", bufs=2) as smallp,
            tc.tile_pool(name="stage", bufs=5) as stagep,
            tc.tile_pool(name="ps", bufs=3, space="PSUM") as psp,
            tc.tile_pool(name="wm", bufs=2, space="PSUM") as wmp,
            tc.tile_pool(name="pt", bufs=3, space="PSUM") as ptp,
            tc.tile_pool(name="dram", bufs=1, space="DRAM") as dramp,
        ):
            # ---- bulk loads: one trigger per blob, split across both DGE rings ----
            cb_s = constp.tile([128, 27], F32, tag="cb")
            nc.scalar.dma_start(out=cb_s[:], in_=cb[:])
            tab2_s = constp.tile([1, 128], BF16, tag="tab2")
            nc.scalar.dma_start(out=tab2_s[:], in_=tab2r[:])
            w2_s = constp.tile([128, 2816], BF16, tag="w2")
            nc.scalar.dma_start(out=w2_s[:], in_=w2b[:])
            w1_s = constp.tile([128, 10240], BF16, tag="w1")
            nc.scalar.dma_start(out=w1_s[:], in_=w1b[:])
            headT_s = constp.tile([128, 4, B], BF16, tag="headT")
            nc.scalar.dma_start(out=headT_s[:], in_=headT[:])
            relT_s = constp.tile([128, 4, BSH], BF16, tag="relT")
            nc.scalar.dma_start(out=relT_s[:], in_=relT[:])
            tailT_s = constp.tile([128, 4, NPAD], BF16, tag="tailT")
            nc.scalar.dma_start(out=tailT_s[:], in_=tailT[:])
            core2_h = []
            for h in range(2):
                ct = bigp.tile([128, HALF], BF16, tag="big")
                nc.sync.dma_start(out=ct[:], in_=core2[:, h * HALF:(h + 1) * HALF])
                core2_h.append(ct)

            ident = constp.tile([128, 128], F32)
            make_identity(nc, ident[:])
            ones_row = constp.tile([1, 128], BF16, tag="ones")
            nc.vector.memset(ones_row[:], 1.0)

            def cbc(name, w=None):
                o = _CB[name]
                return cb_s[:, o:o + w] if w else cb_s[:, o:o + 1]

            # persistent tiles
            hsT_s = pers.tile([128, B], BF16)          # hs^T (pre-BN), full B
            tsT_s = pers.tile([128, NPAD], BF16)       # ts^T (+bias)
            tamT_s = pers.tile([128, NPAD], BF16)      # tam^T
            taS = pers.tile([128, NPAD], BF16)         # ta in [n, c] tiles
            WmT_all = pers.tile([128, B], BF16)        # gathered Wm^T raw
            intT_all = pers.tile([128, B], BF16)       # gathered inter^T (score lhsT)
            WmT_nb = pers.tile([128, B], BF16)         # BN1-applied, score lhsT

            # ---------- generic 2-layer MLP producing outT [c, nb] ----------
            def mlp2_T(w1name, b1name, w2name, xt_s, x0, nb, out_ap, b2name):
                w1o, w1nk, w1cols = _W1[w1name]
                w2o, w2m, w2cols = _W2[w2name]
                nm = w1cols // 128
                b1 = _CB[b1name]
                h1_t = []
                for m in range(nm):
                    ps = psp.tile([128, nb], F32, tag="ps")
                    for k in range(w1nk):
                        nc.tensor.matmul(
                            ps[:],
                            w1_s[:, w1o + k * w1cols + m * 128:
                                 w1o + k * w1cols + (m + 1) * 128],
                            xt_s[:, k, x0:x0 + nb],
                            start=(k == 0), stop=(k == w1nk - 1))
                    h1 = h1p.tile([128, nb], BF16, tag="h1")
                    nc.scalar.activation(h1[:], ps[:], AF.Relu,
                                         bias=cb_s[:, b1 + m:b1 + m + 1])
                    h1_t.append(h1)
                ps2 = psp.tile([128, nb], F32, tag="ps")
                for m in range(nm):
                    nc.tensor.matmul(
                        ps2[:], w2_s[:, w2o + m * w2cols:w2o + m * w2cols + w2cols],
                        h1_t[m][:], start=(m == 0), stop=(m == nm - 1))
                if b2name is None:
                    nc.scalar.copy(out_ap, ps2[:])
                else:
                    nc.scalar.add(out_ap, ps2[:], cbc(b2name))
                return h1_t

            # ---------------- head MLP, full B ----------------
            for bg in range(B // 512):
                mlp2_T("hsw1", "hsb1", "hsw2", headT_s, bg * 512, 512,
                       hsT_s[:, bg * 512:(bg + 1) * 512], None)

            # ---------------- BN0 stats (full B, local) ----------------
            def bn_scale_shift(xT_ap, nfree, g_ap, b_ap):
                nchunk = nfree // 512
                st = smallp.tile([128, nchunk, 6], F32, tag="sm6")
                for i in range(nchunk):
                    nc.vector.bn_stats(st[:, i, :], xT_ap[:, i * 512:(i + 1) * 512])
                mv = smallp.tile([128, 2], F32, tag="sm2")
                nc.vector.bn_aggr(mv[:], st[:])
                scale = smallp.tile([128, 1], F32, tag="sm1a")
                shift = smallp.tile([128, 1], F32, tag="sm1b")
                tmp = smallp.tile([128, 1], F32, tag="sm1c")
                nc.vector.tensor_scalar_add(tmp[:], mv[:, 1:2], 1e-5)
                nc.scalar.activation(scale[:], tmp[:], AF.Sqrt)
                nc.vector.reciprocal(scale[:], scale[:])
                nc.vector.tensor_mul(scale[:], scale[:], g_ap)
                nc.vector.tensor_mul(tmp[:], mv[:, 0:1], scale[:])
                nc.vector.tensor_sub(shift[:], b_ap, tmp[:])
                return scale, shift

            bn0_scale, bn0_shift = bn_scale_shift(hsT_s[:], B, cbc("bn0g"),
                                                  cbc("bn0b"))

            # ha for my shard (headT is rotated so cols [0:BSH] are my rows)
            haT_aff = smallp.tile([128, BSH], F32, tag="haT")
            nc.vector.tensor_scalar(haT_aff[:], hsT_s[:, 0:BSH], bn0_scale[:, 0:1],
                                    bn0_shift[:, 0:1], op0=ALU.mult, op1=ALU.add)
            ha_t = []
            for t in range(2):
                pst = ptp.tile([128, 128], F32, tag="pt")
                nc.tensor.transpose(pst[:], haT_aff[:, t * 128:(t + 1) * 128], ident[:])
                ha = smallp.tile([128, 128], F32, tag="ha")
                nc.scalar.copy(ha[:], pst[:])
                ha_t.append(ha)

            # ---------------- rel MLP (shard) -> rsT (bf16) ----------------
            rsT_bf = smallp.tile([128, BSH], BF16, tag="rsTbf")
            mlp2_T("rsw1", "rsb1", "rsw2", relT_s, 0, BSH, rsT_bf[:], "rsb2")

            # ---------------- hr MLP (shard) -> hraT -> hrm tiles ----------------
            w1o, _, w1c = _W1["hrw1"]
            hr_h1 = []
            for m in range(2):
                ps = psp.tile([128, BSH], F32, tag="ps")
                for k in range(8):
                    xt = (headT_s[:, k, 0:BSH] if k < 4
                          else relT_s[:, k - 4, :])
                    nc.tensor.matmul(ps[:],
                                     w1_s[:, w1o + k * w1c + m * 128:
                                          w1o + k * w1c + (m + 1) * 128],
                                     xt, start=(k == 0), stop=(k == 7))
                h1 = h1p.tile([128, BSH], BF16, tag="h1")
                nc.scalar.activation(h1[:], ps[:], AF.Relu,
                                     bias=cb_s[:, _CB["hrb1"] + m:_CB["hrb1"] + m + 1])
                hr_h1.append(h1)
            w2o, _, w2c = _W2["hrw2"]
            hr_h2 = []
            for m in range(2):
                ps = psp.tile([128, BSH], F32, tag="ps")
                for k in range(2):
                    nc.tensor.matmul(ps[:],
                                     w2_s[:, w2o + k * w2c + m * 128:
                                          w2o + k * w2c + (m + 1) * 128],
                                     hr_h1[k][:], start=(k == 0), stop=(k == 1))
                h2 = h1p.tile([128, BSH], BF16, tag="h1")
                nc.scalar.activation(h2[:], ps[:], AF.Relu,
                                     bias=cb_s[:, _CB["hrb2"] + m:_CB["hrb2"] + m + 1])
                hr_h2.append(h2)
            w3o, _, _ = _W2["hrw3"]
            hraT = smallp.tile([128, BSH], F32, tag="hraT")
            ps3 = psp.tile([128, BSH], F32, tag="ps")
            for k in range(2):
                nc.tensor.matmul(ps3[:], w2_s[:, w3o + k * 128:w3o + (k + 1) * 128],
                                 hr_h2[k][:], start=(k == 0), stop=(k == 1))
            nc.scalar.add(hraT[:], ps3[:], cbc("hrb3"))

            # ---------- soft top-10 mask helper ----------
            def topk_mask_mul(src_sb, out_ap):
                """out = sigmoid((src - thr10)/TEMP) * src on [128,128]."""
                m8 = smallp.tile([128, 8], BF16, tag="m8")
                zap = smallp.tile([128, 128], BF16, tag="zap")
                nc.vector.max(out=m8[:], in_=src_sb)
                nc.vector.match_replace(out=zap[:], in_to_replace=m8[:],
                                        in_values=src_sb, imm_value=NEG)
                nc.vector.max(out=m8[:], in_=zap[:])
                thr = smallp.tile([128, 1], F32, tag="thr")
                nc.vector.tensor_scalar_mul(thr[:], m8[:, 1:2], -1.0 / TEMP)
                mask = smallp.tile([128, 128], BF16, tag="mask")
                nc.scalar.activation(mask[:], src_sb, AF.Sigmoid,
                                     bias=thr[:, 0:1], scale=1.0 / TEMP)
                nc.vector.tensor_mul(out_ap, mask[:], src_sb)

            hrm_bf = []
            for t in range(2):
                pst = ptp.tile([128, 128], F32, tag="pt")
                nc.tensor.transpose(pst[:], hraT[:, t * 128:(t + 1) * 128], ident[:])
                hra = smallp.tile([128, 128], BF16, tag="hra")
                nc.scalar.copy(hra[:], pst[:])
                hb = smallp.tile([128, 128], BF16, tag="hrmbf")
                topk_mask_mul(hra[:], hb[:])
                hrm_bf.append(hb)

            # ---------------- Wm (shard): V blocks + stt accumulation ----------------
            acc_bf = []
            for t in range(2):
                acc = smallp.tile([128, 128], F32, tag="wacc")
                for blk in range(C * C // 512):
                    hsel, hblk = divmod(blk, 16)
                    ps = wmp.tile([128, 512], F32, tag="wps")
                    nc.tensor.matmul(ps[:], rsT_bf[:, t * 128:(t + 1) * 128],
                                     core2_h[hsel][:, hblk * 512:(hblk + 1) * 512],
                                     start=True, stop=True)
                    for j in range(4):
                        cidx = blk * 4 + j
                        if cidx == 0:
                            nc.vector.tensor_scalar(
                                acc[:], ps[:, j * 128:(j + 1) * 128],
                                ha_t[t][:, 0:1], None, op0=ALU.mult)
                        else:
                            nc.vector.scalar_tensor_tensor(
                                acc[:], ps[:, j * 128:(j + 1) * 128],
                                ha_t[t][:, cidx:cidx + 1], acc[:],
                                op0=ALU.mult, op1=ALU.add)
                ab = smallp.tile([128, 128], BF16, tag="waccb")
                nc.vector.tensor_copy(ab[:], acc[:])
                acc_bf.append(ab)

            # ------- tail MLPs (shard, 5 groups of 512), masks deferred -------
            taw1o, _, taw1c = _W1["taw1"]
            taw2o, _, _ = _W2["taw2"]
            for g in range(NG):
                mlp2_T("tsw1", "tsb1", "tsw2", tailT_s, g * 512, 512,
                       tsT_s[:, g * 512:(g + 1) * 512], "tsb2")
                h1_ta = []
                for m in range(4):
                    ps = psp.tile([128, 512], F32, tag="ps")
                    for k in range(4):
                        nc.tensor.matmul(
                            ps[:],
                            w1_s[:, taw1o + k * taw1c + m * 128:
                                 taw1o + k * taw1c + (m + 1) * 128],
                            tailT_s[:, k, g * 512:(g + 1) * 512],
                            start=(k == 0), stop=(k == 3))
                    h1 = h1p.tile([128, 512], BF16, tag="h1")
                    nc.scalar.activation(h1[:], ps[:], AF.Relu,
                                         bias=cb_s[:, _CB["tab1"] + m:
                                                   _CB["tab1"] + m + 1])
                    h1_ta.append(h1)
                for j in range(4):
                    ps_ta = ptp.tile([128, 128], F32, tag="pt")
                    for m in range(4):
                        nc.tensor.matmul(ps_ta[:],
                                         h1_ta[m][:, j * 128:(j + 1) * 128],
                                         w2_s[:, taw2o + m * 128:taw2o + (m + 1) * 128],
                                         start=(m == 0), stop=False)
                    nc.tensor.matmul(ps_ta[:], ones_row[:], tab2_s[:],
                                     start=False, stop=True)
                    nc.scalar.copy(
                        taS[:, g * 512 + j * 128:g * 512 + (j + 1) * 128], ps_ta[:])

            # ---------------- inter: pc load + fused multiply-reduce ----------------
            # last (t,h) unit runs on gpsimd to shorten the DVE serial chain
            intr0 = smallp.tile([128, 128], F32, tag="intr0")
            intr1 = smallp.tile([128, 128], F32, tag="intr1")
            intr_t = [intr0, intr1]
            iscr = smallp.tile([128, 128], BF16, tag="iscr")
            iscr2 = smallp.tile([128, 128], BF16, tag="iscr2")
            intr_bf = []
            for t in range(2):
                for h in range(2):
                    pc = bigp.tile([128, HALF], BF16, tag="big")
                    nc.sync.dma_start(out=pc[:], in_=pcg[t, h])
                    pc3 = pc[:].rearrange("p (d c) -> p d c", c=128)
                    for d in range(64):
                        nc.vector.scalar_tensor_tensor(
                            out=iscr[:], in0=pc3[:, d, :], scalar=1.0,
                            in1=hrm_bf[t][:], op0=ALU.mult, op1=ALU.mult,
                            accum_out=intr_t[t][:, h * 64 + d:h * 64 + d + 1])
                ib = smallp.tile([128, 128], BF16, tag="intrb")
                nc.vector.tensor_copy(ib[:], intr_t[t][:])
                intr_bf.append(ib)

            # ---------------- AllGathers (DMA-xbar transposes, off the PE queue) ----
            WmT_sh = smallp.tile([128, BSH], BF16, tag="WmTsh")
            for t in range(2):
                nc.sync.dma_start_transpose(WmT_sh[:, t * 128:(t + 1) * 128],
                                            acc_bf[t][:])
            ag_w_in = dramp.tile([128, BSH], BF16)
            ag_w_out = dramp.tile([NCORES, 128, BSH], BF16, addr_space="Shared")
            nc.sync.dma_start(out=ag_w_in[:], in_=WmT_sh[:])
            nc.gpsimd.collective_compute(
                "AllGather", ALU.bypass,
                replica_groups=[list(range(NCORES))],
                ins=[ag_w_in.opt()], outs=[ag_w_out.opt()])
            nc.sync.dma_start(out=WmT_all[:],
                              in_=ag_w_out[:].rearrange("r d b -> d r b"))

            intT_sh = smallp.tile([128, BSH], BF16, tag="intTsh")
            for t in range(2):
                nc.sync.dma_start_transpose(intT_sh[:, t * 128:(t + 1) * 128],
                                            intr_bf[t][:])
            ag_i_in = dramp.tile([128, BSH], BF16)
            ag_i_out = dramp.tile([NCORES, 128, BSH], BF16, addr_space="Shared")
            nc.sync.dma_start(out=ag_i_in[:], in_=intT_sh[:])
            nc.gpsimd.collective_compute(
                "AllGather", ALU.bypass,
                replica_groups=[list(range(NCORES))],
                ins=[ag_i_in.opt()], outs=[ag_i_out.opt()])
            nc.sync.dma_start(out=intT_all[:],
                              in_=ag_i_out[:].rearrange("r d b -> d r b"))

            # BN1 on gathered WmT (full B)
            bn1_scale, bn1_shift = bn_scale_shift(WmT_all[:], B, cbc("bn1g"),
                                                  cbc("bn1b"))
            nc.vector.tensor_scalar(WmT_nb[:], WmT_all[:], bn1_scale[:, 0:1],
                                    bn1_shift[:, 0:1], op0=ALU.mult, op1=ALU.add)

            # ---------------- deferred topk masks (tamT, poss score rhs) ------------
            for i in range(NPAD // 128):
                tam_nt = smallp.tile([128, 128], F32, tag="tamnt")
                topk_mask_mul(taS[:, i * 128:(i + 1) * 128], tam_nt[:])
                pst2 = ptp.tile([128, 128], F32, tag="pt")
                nc.tensor.transpose(pst2[:], tam_nt[:], ident[:])
                nc.scalar.copy(tamT_s[:, i * 128:(i + 1) * 128], pst2[:])

            # ---------------- scores: tucker then poss, staged DMA out --------------
            def score_pass(lhsT_all, rhsT, out_dram):
                for bt in range(NB_FULL):
                    st = stagep.tile([128, NPAD], BF16, tag="sst")
                    for g in range(NG):
                        ps_t = psp.tile([128, 512], F32, tag="ps")
                        nc.tensor.matmul(ps_t[:], lhsT_all[:, bt * 128:(bt + 1) * 128],
                                         rhsT[:, g * 512:(g + 1) * 512],
                                         start=True, stop=True)
                        if g % 2 == 0:
                            nc.scalar.copy(st[:, g * 512:(g + 1) * 512], ps_t[:])
                        else:
                            nc.vector.tensor_copy(st[:, g * 512:(g + 1) * 512],
                                                  ps_t[:])
                    nc.sync.dma_start(
                        out=out_dram[bt * 128:(bt + 1) * 128, :],
                        in_=st[:, 0:NSH])

            score_pass(WmT_nb, tsT_s, tucker)
            score_pass(intT_all, tamT_s, poss)

    nc.finalize()
    return nc


# ---------------------------------------------------------------------------
# host side
# ---------------------------------------------------------------------------

def _to_np(x, dt=np.float32):
    return np.ascontiguousarray(np.asarray(x), dtype=dt)


def prepare_in_maps(inputs):
    bf = np.dtype(ml_dtypes.bfloat16)
    head = _to_np(inputs["head_vector"])        # [B, E]
    rel = _to_np(inputs["relation_vector"])     # [B, E]
    ridx = np.ascontiguousarray(np.asarray(inputs["relation_index"]).astype(np.int64))
    tailv = _to_np(inputs["tail_vector"])       # [N, E]
    codebook = _to_np(inputs["codebook"])       # [R2, C, C]
    core = _to_np(inputs["core"])               # [C, C, C]

    cbt2 = np.tanh(codebook).transpose(0, 2, 1).reshape(R2, 2, HALF).astype(bf)
    core2_host = np.ascontiguousarray(core.reshape(C, C * C)).astype(bf)

    def kchunks(xT):                            # [E, cols] -> [128, nk, cols]
        return np.ascontiguousarray(
            xT.reshape(xT.shape[0] // 128, 128, xT.shape[1]).transpose(1, 0, 2))

    headT = kchunks(head.T.astype(bf))          # [128, 4, B]
    relT_full = rel.T.astype(bf)                # [E, B]
    tailT_full = tailv.T.astype(bf)             # [E, N]

    # w1 blob [128, 10240]
    w1b = np.zeros((128, 10240), bf)
    for name, (off, nk, cols) in _W1.items():
        w = _to_np(inputs[name]).astype(bf)     # [nk*128, cols]
        for k in range(nk):
            w1b[:, off + k * cols:off + (k + 1) * cols] = w[k * 128:(k + 1) * 128]
    # w2 blob [128, 2816]: [E, C] viewed as [128, m, cols] flattened
    w2b = np.zeros((128, 2816), bf)
    for name, (off, m, cols) in _W2.items():
        w = _to_np(inputs[name]).astype(bf)     # [m*128, cols]
        v = w.reshape(m, 128, cols).transpose(1, 0, 2).reshape(128, m * cols)
        w2b[:, off:off + m * cols] = v
    # const blob [128, 27] f32
    cbb = np.zeros((128, 27), np.float32)

    def put_chunked(name, src, nk):
        cbb[:, _CB[name]:_CB[name] + nk] = _to_np(src).reshape(nk, 128).T

    put_chunked("hsb1", inputs["hsb1"], 4)
    put_chunked("rsb1", inputs["rsb1"], 4)
    put_chunked("tsb1", inputs["tsb1"], 4)
    put_chunked("tab1", inputs["tab1"], 4)
    put_chunked("hrb1", inputs["hrb1"], 2)
    put_chunked("hrb2", inputs["hrb2"], 2)
    for name, key in [("hrb3", "hrb3"), ("rsb2", "rsb2"), ("tsb2", "tsb2"),
                      ("bn0g", "bn0_g"), ("bn0b", "bn0_b"),
                      ("bn1g", "bn1_g"), ("bn1b", "bn1_b")]:
        cbb[:, _CB[name]] = _to_np(inputs[key])

    weights_common = {
        "w1b": w1b, "w2b": w2b, "cb": cbb,
        "tab2r": _to_np(inputs["tab2"]).reshape(1, 128).astype(bf),
        "core2": core2_host,
    }

    in_maps = []
    for k in range(NCORES):
        b0 = k * BSH
        n0 = k * NSH
        # rotate headT so THIS core's 256 b-columns come first; BN0 stats are
        # order-invariant, and slices [0:256] are "my" shard on every core.
        headT_k = np.ascontiguousarray(np.roll(headT, -b0, axis=2))
        tailT_k = np.zeros((E, NPAD), bf)
        tailT_k[:, :NSH] = tailT_full[:, n0:n0 + NSH]
        pcg_k = np.ascontiguousarray(
            cbt2[ridx[b0:b0 + BSH]].reshape(2, 128, 2, HALF).transpose(0, 2, 1, 3))
        m = dict(weights_common)
        m["headT"] = headT_k
        m["relT"] = kchunks(np.ascontiguousarray(relT_full[:, b0:b0 + BSH]))
        m["tailT"] = kchunks(tailT_k)
        m["pcg"] = pcg_k
        in_maps.append(m)
    return in_maps


def assemble_outputs(results):
    tuckers, posses = [], []
    for k in range(NCORES):
        r = results[k]
        tuckers.append(np.asarray(r["tucker"]).astype(np.float32))
        posses.append(np.asarray(r["poss"]).astype(np.float32))
    tucker_full = np.concatenate(tuckers, axis=1)
    poss_full = np.concatenate(posses, axis=1)
    return tucker_full, poss_full


def kernel(**inputs):
    if "prog" not in _PROG_CACHE:
        _PROG_CACHE["prog"] = build_program()
    nc = _PROG_CACHE["prog"]
    in_maps = prepare_in_maps(inputs)
    res = run_bass_kernel_spmd(nc, in_maps, list(range(NCORES)))
    return assemble_outputs(res.results)


# revision 19
# speedup vs baseline: 1.0141x; 1.0141x over previous
"""Trainium2 Bass kernel for nn_BaseModel_74302934220896 (TuckER + possibility-codebook).

Contract: kernel(**inputs) takes FULL unsharded inputs (as in reference.setup_inputs())
and returns the full output tuple (tucker_logits [B,N] f32, possibility_score [B,N] f32).

Sharding (8 cores):
  - B (2048) -> 8 x 256 for relation / hr / Wm / codebook paths
  - N (20000) -> 8 x 2500 (padded to 2560) for tail features and the [B,N] score matmuls
  - head MLP replicated over full B on every core so BN0 needs no collective
  - codebook is tanh'ed and gathered per-row on the HOST (pcg input)
  - two small bf16 AllGathers (WmT first, interT second); BN1 stats computed locally
    from the gathered full-B WmT.

v4 notes: all weights/activations are host-packed into a handful of big DMA blobs
(one HWDGE trigger each, ~620ns per trigger on the sequencer), loads split across
the SP and ACT DGE rings, AllGather path uses DMA-xbar transposes (off the PE
queue), masks deferred past BN1, score evacuations split DVE/ACT, outputs staged
[128, 2560] bf16 with 5KB-line DMAs.
"""

import sys

sys.path.insert(0, "/opt/trn_rl_repo")

import numpy as np
import ml_dtypes

import concourse.bass as bass
import concourse.bacc as bacc
import concourse.mybir as mybir
import concourse.tile as tile
from concourse.bass_utils import run_bass_kernel_spmd
from concourse.masks import make_identity

F32 = mybir.dt.float32
BF16 = mybir.dt.bfloat16
AF = mybir.ActivationFunctionType
ALU = mybir.AluOpType
AX = mybir.AxisListType

B, N, E, C, R2 = 2048, 20000, 512, 128, 474
NCORES = 8
BSH = B // NCORES            # 256 b rows per core (sharded paths)
NSH = N // NCORES            # 2500 tail rows per core
NPAD = 2560                  # padded to 5 groups of 512
NG = NPAD // 512             # 5 n-groups
NB_FULL = B // 128           # 16 b-tiles over full B
TEMP = 0.5
NEG = -1.0e30
HALF = C * C // 2

# const blob column layout (f32 [128, 27])
_CB = dict(hsb1=0, rsb1=4, tsb1=8, tab1=12, hrb1=16, hrb2=18, hrb3=20, rsb2=21,
           tsb2=22, bn0g=23, bn0b=24, bn1g=25, bn1b=26)
# w2 blob column layout (bf16 [128, 2816]); entries are (offset, m, cols)
_W2 = dict(hsw2=(0, 4, 128), rsw2=(512, 4, 128), tsw2=(1024, 4, 128),
           taw2=(1536, 4, 128), hrw3=(2048, 2, 128), hrw2=(2304, 2, 256))
# w1 blob column layout (bf16 [128, 10240]); entries are (offset, nk, cols)
_W1 = dict(hsw1=(0, 4, 512), rsw1=(2048, 4, 512), tsw1=(4096, 4, 512),
           taw1=(6144, 4, 512), hrw1=(8192, 8, 256))

_PROG_CACHE = {}


def build_program():
    nc = bacc.Bacc("TRN2", target_bir_lowering=False, debug=False,
                   num_devices=NCORES)

    # ---------------- DRAM I/O ----------------
    dI = lambda name, shape, dt=BF16: nc.dram_tensor(name, shape, dt, kind="ExternalInput")
    headT = dI("headT", [128, 4, B])                   # full-B head^T, [p, k, b]
    relT = dI("relT", [128, 4, BSH])
    tailT = dI("tailT", [128, 4, NPAD])
    pcg = dI("pcg", [2, 2, 128, HALF])                 # host-gathered tanh codebook
    core2 = dI("core2", [C, C * C])                    # core reshaped [e, (c,d)]
    w1b = dI("w1b", [128, 10240])
    w2b = dI("w2b", [128, 2816])
    cb = dI("cb", [128, 27], F32)
    tab2r = dI("tab2r", [1, C])

    tucker = nc.dram_tensor("tucker", [B, NSH], BF16, kind="ExternalOutput")
    poss = nc.dram_tensor("poss", [B, NSH], BF16, kind="ExternalOutput")

    with tile.TileContext(nc) as tc:
        with (
            tc.tile_pool(name="const", bufs=1) as constp,
            tc.tile_pool(name="big", bufs=3) as bigp,
            tc.tile_pool(name="h1", bufs=8) as h1p,
            tc.tile_pool(name="pers", bufs=1) as pers,
            tc.tile_pool(name="small", bufs=2) as smallp,
            tc.tile_pool(name="stage", bufs=4) as stagep,
            tc.tile_pool(name="ps", bufs=3, space="PSUM") as psp,
            tc.tile_pool(name="wm", bufs=2, space="PSUM") as wmp,
            tc.tile_pool(name="pt", bufs=3, space="PSUM") as ptp,
            tc.tile_pool(name="dram", bufs=1, space="DRAM") as dramp,
        ):
            # ---- bulk loads: one trigger per blob, split across both DGE rings ----
            cb_s = constp.tile([128, 27], F32, tag="cb")
            nc.scalar.dma_start(out=cb_s[:], in_=cb[:])
            tab2_s = constp.tile([1, 128], BF16, tag="tab2")
            nc.scalar.dma_start(out=tab2_s[:], in_=tab2r[:])
            w1_s = constp.tile([128, 10240], BF16, tag="w1")
            nc.scalar.dma_start(out=w1_s[:], in_=w1b[:])
            headT_s = constp.tile([128, 4, B], BF16, tag="headT")
            nc.scalar.dma_start(out=headT_s[:], in_=headT[:])
            relT_s = constp.tile([128, 4, BSH], BF16, tag="relT")
            nc.scalar.dma_start(out=relT_s[:], in_=relT[:])
            w2_s = constp.tile([128, 2816], BF16, tag="w2")
            nc.scalar.dma_start(out=w2_s[:], in_=w2b[:])
            tailT_s = constp.tile([128, 4, NPAD], BF16, tag="tailT")
            nc.scalar.dma_start(out=tailT_s[:], in_=tailT[:])
            core2_h = []
            for h in range(2):
                ct = bigp.tile([128, HALF], BF16, tag="big")
                nc.sync.dma_start(out=ct[:], in_=core2[:, h * HALF:(h + 1) * HALF])
                core2_h.append(ct)

            ident = constp.tile([128, 128], F32)
            make_identity(nc, ident[:])
            ones_row = constp.tile([1, 128], BF16, tag="ones")
            nc.vector.memset(ones_row[:], 1.0)

            def cbc(name, w=None):
                o = _CB[name]
                return cb_s[:, o:o + w] if w else cb_s[:, o:o + 1]

            # persistent tiles
            hsT_s = pers.tile([128, B], BF16)          # hs^T (pre-BN), full B
            tsT_s = pers.tile([128, NPAD], BF16)       # ts^T (+bias)
            tamT_s = pers.tile([128, NPAD], BF16)      # tam^T
            taS = pers.tile([128, NPAD], BF16)         # ta in [n, c] tiles
            WmT_all = pers.tile([128, B], BF16)        # gathered Wm^T raw
            intT_all = pers.tile([128, B], BF16)       # gathered inter^T (score lhsT)
            WmT_nb = pers.tile([128, B], BF16)         # BN1-applied, score lhsT

            # ---------- generic 2-layer MLP producing outT [c, nb] ----------
            def mlp2_T(w1name, b1name, w2name, xt_s, x0, nb, out_ap, b2name):
                w1o, w1nk, w1cols = _W1[w1name]
                w2o, w2m, w2cols = _W2[w2name]
                nm = w1cols // 128
                b1 = _CB[b1name]
                h1_t = []
                for m in range(nm):
                    ps = psp.tile([128, nb], F32, tag="ps")
                    for k in range(w1nk):
                        nc.tensor.matmul(
                            ps[:],
                            w1_s[:, w1o + k * w1cols + m * 128:
                                 w1o + k * w1cols + (m + 1) * 128],
                            xt_s[:, k, x0:x0 + nb],
                            start=(k == 0), stop=(k == w1nk - 1))
                    h1 = h1p.tile([128, nb], BF16, tag="h1")
                    nc.scalar.activation(h1[:], ps[:], AF.Relu,
                                         bias=cb_s[:, b1 + m:b1 + m + 1])
                    h1_t.append(h1)
                ps2 = psp.tile([128, nb], F32, tag="ps")
                for m in range(nm):
                    nc.tensor.matmul(
                        ps2[:], w2_s[:, w2o + m * w2cols:w2o + m * w2cols + w2cols],
                        h1_t[m][:], start=(m == 0), stop=(m == nm - 1))
                if b2name is None:
                    nc.scalar.copy(out_ap, ps2[:])
                else:
                    nc.scalar.add(out_ap, ps2[:], cbc(b2name))
                return h1_t

            # ---------------- head MLP, full B ----------------
            for bg in range(B // 512):
                mlp2_T("hsw1", "hsb1", "hsw2", headT_s, bg * 512, 512,
                       hsT_s[:, bg * 512:(bg + 1) * 512], None)

            # ---------------- BN0 stats (full B, local) ----------------
            def bn_scale_shift(xT_ap, nfree, g_ap, b_ap):
                nchunk = nfree // 512
                st = smallp.tile([128, nchunk, 6], F32, tag="sm6")
                for i in range(nchunk):
                    nc.vector.bn_stats(st[:, i, :], xT_ap[:, i * 512:(i + 1) * 512])
                mv = smallp.tile([128, 2], F32, tag="sm2")
                nc.vector.bn_aggr(mv[:], st[:])
                scale = smallp.tile([128, 1], F32, tag="sm1a")
                shift = smallp.tile([128, 1], F32, tag="sm1b")
                tmp = smallp.tile([128, 1], F32, tag="sm1c")
                nc.vector.tensor_scalar_add(tmp[:], mv[:, 1:2], 1e-5)
                nc.scalar.activation(scale[:], tmp[:], AF.Sqrt)
                nc.vector.reciprocal(scale[:], scale[:])
                nc.vector.tensor_mul(scale[:], scale[:], g_ap)
                nc.vector.tensor_mul(tmp[:], mv[:, 0:1], scale[:])
                nc.vector.tensor_sub(shift[:], b_ap, tmp[:])
                return scale, shift

            bn0_scale, bn0_shift = bn_scale_shift(hsT_s[:], B, cbc("bn0g"),
                                                  cbc("bn0b"))

            # ha for my shard (headT is rotated so cols [0:BSH] are my rows)
            haT_aff = smallp.tile([128, BSH], F32, tag="haT")
            nc.vector.tensor_scalar(haT_aff[:], hsT_s[:, 0:BSH], bn0_scale[:, 0:1],
                                    bn0_shift[:, 0:1], op0=ALU.mult, op1=ALU.add)
            ha_t = []
            for t in range(2):
                pst = ptp.tile([128, 128], F32, tag="pt")
                nc.tensor.transpose(pst[:], haT_aff[:, t * 128:(t + 1) * 128], ident[:])
                ha = smallp.tile([128, 128], F32, tag="ha")
                nc.scalar.copy(ha[:], pst[:])
                ha_t.append(ha)

            # ---------------- rel MLP (shard) -> rsT (bf16) ----------------
            rsT_bf = smallp.tile([128, BSH], BF16, tag="rsTbf")
            mlp2_T("rsw1", "rsb1", "rsw2", relT_s, 0, BSH, rsT_bf[:], "rsb2")

            # ---------------- hr MLP (shard) -> hraT -> hrm tiles ----------------
            w1o, _, w1c = _W1["hrw1"]
            hr_h1 = []
            for m in range(2):
                ps = psp.tile([128, BSH], F32, tag="ps")
                for k in range(8):
                    xt = (headT_s[:, k, 0:BSH] if k < 4
                          else relT_s[:, k - 4, :])
                    nc.tensor.matmul(ps[:],
                                     w1_s[:, w1o + k * w1c + m * 128:
                                          w1o + k * w1c + (m + 1) * 128],
                                     xt, start=(k == 0), stop=(k == 7))
                h1 = h1p.tile([128, BSH], BF16, tag="h1")
                nc.scalar.activation(h1[:], ps[:], AF.Relu,
                                     bias=cb_s[:, _CB["hrb1"] + m:_CB["hrb1"] + m + 1])
                hr_h1.append(h1)
            w2o, _, w2c = _W2["hrw2"]
            hr_h2 = []
            for m in range(2):
                ps = psp.tile([128, BSH], F32, tag="ps")
                for k in range(2):
                    nc.tensor.matmul(ps[:],
                                     w2_s[:, w2o + k * w2c + m * 128:
                                          w2o + k * w2c + (m + 1) * 128],
                                     hr_h1[k][:], start=(k == 0), stop=(k == 1))
                h2 = h1p.tile([128, BSH], BF16, tag="h1")
                nc.scalar.activation(h2[:], ps[:], AF.Relu,
                                     bias=cb_s[:, _CB["hrb2"] + m:_CB["hrb2"] + m + 1])
                hr_h2.append(h2)
            w3o, _, _ = _W2["hrw3"]
            hraT = smallp.tile([128, BSH], F32, tag="hraT")
            ps3 = psp.tile([128, BSH], F32, tag="ps")
            for k in range(2):
                nc.tensor.matmul(ps3[:], w2_s[:, w3o + k * 128:w3o + (k + 1) * 128],
                                 hr_h2[k][:], start=(k == 0), stop=(k == 1))
            nc.scalar.add(hraT[:], ps3[:], cbc("hrb3"))

            # ---------- soft top-10 mask helper ----------
            def topk_mask_mul(src_sb, out_ap):
                """out = sigmoid((src - thr10)/TEMP) * src on [128,128]."""
                m8 = smallp.tile([128, 8], BF16, tag="m8")
                zap = smallp.tile([128, 128], BF16, tag="zap")
                nc.vector.max(out=m8[:], in_=src_sb)
                nc.vector.match_replace(out=zap[:], in_to_replace=m8[:],
                                        in_values=src_sb, imm_value=NEG)
                nc.vector.max(out=m8[:], in_=zap[:])
                thr = smallp.tile([128, 1], F32, tag="thr")
                nc.vector.tensor_scalar_mul(thr[:], m8[:, 1:2], -1.0 / TEMP)
                mask = smallp.tile([128, 128], BF16, tag="mask")
                nc.scalar.activation(mask[:], src_sb, AF.Sigmoid,
                                     bias=thr[:, 0:1], scale=1.0 / TEMP)
                nc.vector.tensor_mul(out_ap, mask[:], src_sb)

            hrm_bf = []
            for t in range(2):
                pst = ptp.tile([128, 128], F32, tag="pt")
                nc.tensor.transpose(pst[:], hraT[:, t * 128:(t + 1) * 128], ident[:])
                hra = smallp.tile([128, 128], BF16, tag="hra")
                nc.scalar.copy(hra[:], pst[:])
                hb = smallp.tile([128, 128], BF16, tag="hrmbf")
                topk_mask_mul(hra[:], hb[:])
                hrm_bf.append(hb)

            # ---------------- Wm (shard): V blocks + stt accumulation ----------------
            acc_bf = []
            for t in range(2):
                acc = smallp.tile([128, 128], F32, tag="wacc")
                for blk in range(C * C // 512):
                    hsel, hblk = divmod(blk, 16)
                    ps = wmp.tile([128, 512], F32, tag="wps")
                    nc.tensor.matmul(ps[:], rsT_bf[:, t * 128:(t + 1) * 128],
                                     core2_h[hsel][:, hblk * 512:(hblk + 1) * 512],
                                     start=True, stop=True)
                    for j in range(4):
                        cidx = blk * 4 + j
                        if cidx == 0:
                            nc.vector.tensor_scalar(
                                acc[:], ps[:, j * 128:(j + 1) * 128],
                                ha_t[t][:, 0:1], None, op0=ALU.mult)
                        else:
                            nc.vector.scalar_tensor_tensor(
                                acc[:], ps[:, j * 128:(j + 1) * 128],
                                ha_t[t][:, cidx:cidx + 1], acc[:],
                                op0=ALU.mult, op1=ALU.add)
                ab = smallp.tile([128, 128], BF16, tag="waccb")
                nc.vector.tensor_copy(ab[:], acc[:])
                acc_bf.append(ab)

            # ------- tail MLPs (shard, 5 groups of 512), masks deferred -------
            taw1o, _, taw1c = _W1["taw1"]
            taw2o, _, _ = _W2["taw2"]
            for g in range(NG):
                mlp2_T("tsw1", "tsb1", "tsw2", tailT_s, g * 512, 512,
                       tsT_s[:, g * 512:(g + 1) * 512], "tsb2")
                h1_ta = []
                for m in range(4):
                    ps = psp.tile([128, 512], F32, tag="ps")
                    for k in range(4):
                        nc.tensor.matmul(
                            ps[:],
                            w1_s[:, taw1o + k * taw1c + m * 128:
                                 taw1o + k * taw1c + (m + 1) * 128],
                            tailT_s[:, k, g * 512:(g + 1) * 512],
                            start=(k == 0), stop=(k == 3))
                    h1 = h1p.tile([128, 512], BF16, tag="h1")
                    nc.scalar.activation(h1[:], ps[:], AF.Relu,
                                         bias=cb_s[:, _CB["tab1"] + m:
                                                   _CB["tab1"] + m + 1])
                    h1_ta.append(h1)
                for j in range(4):
                    ps_ta = ptp.tile([128, 128], F32, tag="pt")
                    for m in range(4):
                        nc.tensor.matmul(ps_ta[:],
                                         h1_ta[m][:, j * 128:(j + 1) * 128],
                                         w2_s[:, taw2o + m * 128:taw2o + (m + 1) * 128],
                                         start=(m == 0), stop=False)
                    nc.tensor.matmul(ps_ta[:], ones_row[:], tab2_s[:],
                                     start=False, stop=True)
                    nc.scalar.copy(
                        taS[:, g * 512 + j * 128:g * 512 + (j + 1) * 128], ps_ta[:])

            # ---------------- inter: pc load + fused multiply-reduce ----------------
            # last (t,h) unit runs on gpsimd to shorten the DVE serial chain
            intr0 = smallp.tile([128, 128], F32, tag="intr0")
            intr1 = smallp.tile([128, 128], F32, tag="intr1")
            intr_t = [intr0, intr1]
            iscr = smallp.tile([128, 128], BF16, tag="iscr")
            iscr2 = smallp.tile([128, 128], BF16, tag="iscr2")
            intr_bf = []
            for t in range(2):
                for h in range(2):
                    pc = bigp.tile([128, HALF], BF16, tag="big")
                    nc.sync.dma_start(out=pc[:], in_=pcg[t, h])
                    pc3 = pc[:].rearrange("p (d c) -> p d c", c=128)
                    for d in range(64):
                        nc.vector.scalar_tensor_tensor(
                            out=iscr[:], in0=pc3[:, d, :], scalar=1.0,
                            in1=hrm_bf[t][:], op0=ALU.mult, op1=ALU.mult,
                            accum_out=intr_t[t][:, h * 64 + d:h * 64 + d + 1])
                ib = smallp.tile([128, 128], BF16, tag="intrb")
                nc.vector.tensor_copy(ib[:], intr_t[t][:])
                intr_bf.append(ib)

            # ---------------- AllGathers (DMA-xbar transposes, off the PE queue) ----
            WmT_sh = smallp.tile([128, BSH], BF16, tag="WmTsh")
            for t in range(2):
                nc.sync.dma_start_transpose(WmT_sh[:, t * 128:(t + 1) * 128],
                                            acc_bf[t][:])
            ag_w_in = dramp.tile([128, BSH], BF16)
            ag_w_out = dramp.tile([NCORES, 128, BSH], BF16, addr_space="Shared")
            nc.sync.dma_start(out=ag_w_in[:], in_=WmT_sh[:])
            nc.gpsimd.collective_compute(
                "AllGather", ALU.bypass,
                replica_groups=[list(range(NCORES))],
                ins=[ag_w_in.opt()], outs=[ag_w_out.opt()])
            nc.sync.dma_start(out=WmT_all[:],
                              in_=ag_w_out[:].rearrange("r d b -> d r b"))

            # inter AG chain rides the ACT DGE ring so it cannot block the sync ring
            intT_sh = smallp.tile([128, BSH], BF16, tag="intTsh")
            for t in range(2):
                nc.scalar.dma_start_transpose(intT_sh[:, t * 128:(t + 1) * 128],
                                              intr_bf[t][:])
            ag_i_in = dramp.tile([128, BSH], BF16)
            ag_i_out = dramp.tile([NCORES, 128, BSH], BF16, addr_space="Shared")
            nc.scalar.dma_start(out=ag_i_in[:], in_=intT_sh[:])
            nc.gpsimd.collective_compute(
                "AllGather", ALU.bypass,
                replica_groups=[list(range(NCORES))],
                ins=[ag_i_in.opt()], outs=[ag_i_out.opt()])
            nc.scalar.dma_start(out=intT_all[:],
                                in_=ag_i_out[:].rearrange("r d b -> d r b"))

            # BN1 on gathered WmT (full B)
            bn1_scale, bn1_shift = bn_scale_shift(WmT_all[:], B, cbc("bn1g"),
                                                  cbc("bn1b"))
            nc.vector.tensor_scalar(WmT_nb[:], WmT_all[:], bn1_scale[:, 0:1],
                                    bn1_shift[:, 0:1], op0=ALU.mult, op1=ALU.add)

            # ---------------- deferred topk masks (tamT, poss score rhs) ------------
            for i in range(NPAD // 128):
                tam_nt = smallp.tile([128, 128], F32, tag="tamnt")
                topk_mask_mul(taS[:, i * 128:(i + 1) * 128], tam_nt[:])
                pst2 = ptp.tile([128, 128], F32, tag="pt")
                nc.tensor.transpose(pst2[:], tam_nt[:], ident[:])
                nc.scalar.copy(tamT_s[:, i * 128:(i + 1) * 128], pst2[:])

            # ---------------- scores: tucker then poss, staged DMA out --------------
            def score_pass(lhsT_all, rhsT, out_dram):
                for bt in range(NB_FULL):
                    st = stagep.tile([128, NPAD], BF16, tag="sst")
                    for g in range(NG):
                        ps_t = psp.tile([128, 512], F32, tag="ps")
                        nc.tensor.matmul(ps_t[:], lhsT_all[:, bt * 128:(bt + 1) * 128],
                                         rhsT[:, g * 512:(g + 1) * 512],
                                         start=True, stop=True)
                        if g % 2 == 0:
                            nc.scalar.copy(st[:, g * 512:(g + 1) * 512], ps_t[:])
                        else:
                            nc.vector.tensor_copy(st[:, g * 512:(g + 1) * 512],
                                                  ps_t[:])
                    nc.sync.dma_start(
                        out=out_dram[bt * 128:(bt + 1) * 128, :],
                        in_=st[:, 0:NSH])

            score_pass(WmT_nb, tsT_s, tucker)
            score_pass(intT_all, tamT_s, poss)

    nc.finalize()
    return nc


# ---------------------------------------------------------------------------
# host side
# ---------------------------------------------------------------------------

def _to_np(x, dt=np.float32):
    return np.ascontiguousarray(np.asarray(x), dtype=dt)


def prepare_in_maps(inputs):
    bf = np.dtype(ml_dtypes.bfloat16)
    head = _to_np(inputs["head_vector"])        # [B, E]
    rel = _to_np(inputs["relation_vector"])     # [B, E]
    ridx = np.ascontiguousarray(np.asarray(inputs["relation_index"]).astype(np.int64))
    tailv = _to_np(inputs["tail_vector"])       # [N, E]
    codebook = _to_np(inputs["codebook"])       # [R2, C, C]
    core = _to_np(inputs["core"])               # [C, C, C]

    cbt2 = np.tanh(codebook).transpose(0, 2, 1).reshape(R2, 2, HALF).astype(bf)
    core2_host = np.ascontiguousarray(core.reshape(C, C * C)).astype(bf)

    def kchunks(xT):                            # [E, cols] -> [128, nk, cols]
        return np.ascontiguousarray(
            xT.reshape(xT.shape[0] // 128, 128, xT.shape[1]).transpose(1, 0, 2))

    headT = kchunks(head.T.astype(bf))          # [128, 4, B]
    relT_full = rel.T.astype(bf)                # [E, B]
    tailT_full = tailv.T.astype(bf)             # [E, N]

    # w1 blob [128, 10240]
    w1b = np.zeros((128, 10240), bf)
    for name, (off, nk, cols) in _W1.items():
        w = _to_np(inputs[name]).astype(bf)     # [nk*128, cols]
        for k in range(nk):
            w1b[:, off + k * cols:off + (k + 1) * cols] = w[k * 128:(k + 1) * 128]
    # w2 blob [128, 2816]: [E, C] viewed as [128, m, cols] flattened
    w2b = np.zeros((128, 2816), bf)
    for name, (off, m, cols) in _W2.items():
        w = _to_np(inputs[name]).astype(bf)     # [m*128, cols]
        v = w.reshape(m, 128, cols).transpose(1, 0, 2).reshape(128, m * cols)
        w2b[:, off:off + m * cols] = v
    # const blob [128, 27] f32
    cbb = np.zeros((128, 27), np.float32)

    def put_chunked(name, src, nk):
        cbb[:, _CB[name]:_CB[name] + nk] = _to_np(src).reshape(nk, 128).T

    put_chunked("hsb1", inputs["hsb1"], 4)
    put_chunked("rsb1", inputs["rsb1"], 4)
    put_chunked("tsb1", inputs["tsb1"], 4)
    put_chunked("tab1", inputs["tab1"], 4)
    put_chunked("hrb1", inputs["hrb1"], 2)
    put_chunked("hrb2", inputs["hrb2"], 2)
    for name, key in [("hrb3", "hrb3"), ("rsb2", "rsb2"), ("tsb2", "tsb2"),
                      ("bn0g", "bn0_g"), ("bn0b", "bn0_b"),
                      ("bn1g", "bn1_g"), ("bn1b", "bn1_b")]:
        cbb[:, _CB[name]] = _to_np(inputs[key])

    weights_common = {
        "w1b": w1b, "w2b": w2b, "cb": cbb,
        "tab2r": _to_np(inputs["tab2"]).reshape(1, 128).astype(bf),
        "core2": core2_host,
    }

    in_maps = []
    for k in range(NCORES):
        b0 = k * BSH
        n0 = k * NSH
        # rotate headT so THIS core's 256 b-columns come first; BN0 stats are
        # order-invariant, and slices [0:256] are "my" shard on every core.
        headT_k = np.ascontiguousarray(np.roll(headT, -b0, axis=2))
        tailT_k = np.zeros((E, NPAD), bf)
        tailT_k[:, :NSH] = tailT_full[:, n0:n0 + NSH]
        pcg_k = np.ascontiguousarray(
            cbt2[ridx[b0:b0 + BSH]].reshape(2, 128, 2, HALF).transpose(0, 2, 1, 3))
        m = dict(weights_common)
        m["headT"] = headT_k
        m["relT"] = kchunks(np.ascontiguousarray(relT_full[:, b0:b0 + BSH]))
        m["tailT"] = kchunks(tailT_k)
        m["pcg"] = pcg_k
        in_maps.append(m)
    return in_maps


def assemble_outputs(results):
    tuckers, posses = [], []
    for k in range(NCORES):
        r = results[k]
        tuckers.append(np.asarray(r["tucker"]).astype(np.float32))
        posses.append(np.asarray(r["poss"]).astype(np.float32))
    tucker_full = np.concatenate(tuckers, axis=1)
    poss_full = np.concatenate(posses, axis=1)
    return tucker_full, poss_full


def kernel(**inputs):
    if "prog" not in _PROG_CACHE:
        _PROG_CACHE["prog"] = build_program()
    nc = _PROG_CACHE["prog"]
    in_maps = prepare_in_maps(inputs)
    res = run_bass_kernel_spmd(nc, in_maps, list(range(NCORES)))
    return assemble_outputs(res.results)


# revision 26
# speedup vs baseline: 1.0488x; 1.0342x over previous
"""Trainium2 Bass kernel for nn_BaseModel_74302934220896 (TuckER + possibility-codebook).

Contract: kernel(**inputs) takes FULL unsharded inputs (as in reference.setup_inputs())
and returns the full output tuple (tucker_logits [B,N] f32, possibility_score [B,N] f32).

Sharding (8 cores):
  - B (2048) -> 8 x 256 for relation / hr / Wm / codebook paths
  - N (20000) -> 8 x 2500 (padded to 2560) for tail features and the [B,N] score matmuls
  - head MLP replicated over full B on every core so BN0 needs no collective
  - codebook is tanh'ed and gathered per-row on the HOST (pcg input)
  - two small bf16 AllGathers (WmT first, interT second); BN1 stats computed locally
    from the gathered full-B WmT.

v4 notes: all weights/activations are host-packed into a handful of big DMA blobs
(one HWDGE trigger each, ~620ns per trigger on the sequencer), loads split across
the SP and ACT DGE rings, AllGather path uses DMA-xbar transposes (off the PE
queue), masks deferred past BN1, score evacuations split DVE/ACT, outputs staged
[128, 2560] bf16 with 5KB-line DMAs.
"""

import sys

sys.path.insert(0, "/opt/trn_rl_repo")

import numpy as np
import ml_dtypes

import concourse.bass as bass
import concourse.bacc as bacc
import concourse.mybir as mybir
import concourse.tile as tile
from concourse.bass_utils import run_bass_kernel_spmd
from concourse.masks import make_identity

F32 = mybir.dt.float32
BF16 = mybir.dt.bfloat16
AF = mybir.ActivationFunctionType
ALU = mybir.AluOpType
AX = mybir.AxisListType

B, N, E, C, R2 = 2048, 20000, 512, 128, 474
NCORES = 8
BSH = B // NCORES            # 256 b rows per core (sharded paths)
NSH = N // NCORES            # 2500 tail rows per core
NPAD = 2560                  # padded to 5 groups of 512
NG = NPAD // 512             # 5 n-groups
NB_FULL = B // 128           # 16 b-tiles over full B
TEMP = 0.5
NEG = -1.0e30
HALF = C * C // 2

# const blob column layout (f32 [128, 27])
_CB = dict(hsb1=0, rsb1=4, tsb1=8, tab1=12, hrb1=16, hrb2=18, hrb3=20, rsb2=21,
           tsb2=22, bn0g=23, bn0b=24, bn1g=25, bn1b=26)
# w2 blob column layout (bf16 [128, 2816]); entries are (offset, m, cols)
_W2 = dict(hsw2=(0, 4, 128), rsw2=(512, 4, 128), tsw2=(1024, 4, 128),
           taw2=(1536, 4, 128), hrw3=(2048, 2, 128), hrw2=(2304, 2, 256))
# w1 blob column layout (bf16 [128, 10240]); entries are (offset, nk, cols)
_W1 = dict(hsw1=(0, 4, 512), rsw1=(2048, 4, 512), tsw1=(4096, 4, 512),
           taw1=(6144, 4, 512), hrw1=(8192, 8, 256))

_PROG_CACHE = {}


def build_program():
    nc = bacc.Bacc("TRN2", target_bir_lowering=False, debug=False,
                   num_devices=NCORES)

    # ---------------- DRAM I/O ----------------
    dI = lambda name, shape, dt=BF16: nc.dram_tensor(name, shape, dt, kind="ExternalInput")
    headT = dI("headT", [128, 4, BSH])                 # sharded head^T, [p, k, b]
    relT = dI("relT", [128, 4, BSH])
    tailT = dI("tailT", [128, 4, NPAD])
    pcg = dI("pcg", [2, 2, 128, HALF])                 # host-gathered tanh codebook
    core2 = dI("core2", [C, C * C])                    # core reshaped [e, (c,d)]
    w1b = dI("w1b", [128, 10240])
    w2b = dI("w2b", [128, 2816])
    cb = dI("cb", [128, 27], F32)
    tab2r = dI("tab2r", [1, C])

    tucker = nc.dram_tensor("tucker", [B, NSH], BF16, kind="ExternalOutput")
    poss = nc.dram_tensor("poss", [B, NSH], BF16, kind="ExternalOutput")

    with tile.TileContext(nc) as tc:
        with (
            tc.tile_pool(name="const", bufs=1) as constp,
            tc.tile_pool(name="big", bufs=3) as bigp,
            tc.tile_pool(name="h1", bufs=8) as h1p,
            tc.tile_pool(name="pers", bufs=1) as pers,
            tc.tile_pool(name="small", bufs=2) as smallp,
            tc.tile_pool(name="stage", bufs=4) as stagep,
            tc.tile_pool(name="ps", bufs=3, space="PSUM") as psp,
            tc.tile_pool(name="wm", bufs=2, space="PSUM") as wmp,
            tc.tile_pool(name="pt", bufs=3, space="PSUM") as ptp,
            tc.tile_pool(name="dram", bufs=1, space="DRAM") as dramp,
        ):
            # ---- bulk loads: one trigger per blob, split across both DGE rings ----
            cb_s = constp.tile([128, 27], F32, tag="cb")
            nc.scalar.dma_start(out=cb_s[:], in_=cb[:])
            tab2_s = constp.tile([1, 128], BF16, tag="tab2")
            nc.scalar.dma_start(out=tab2_s[:], in_=tab2r[:])
            w1_s = constp.tile([128, 10240], BF16, tag="w1")
            nc.scalar.dma_start(out=w1_s[:, 0:4096], in_=w1b[:, 0:4096])
            headT_s = constp.tile([128, 4, BSH], BF16, tag="headT")
            nc.scalar.dma_start(out=headT_s[:], in_=headT[:])
            relT_s = constp.tile([128, 4, BSH], BF16, tag="relT")
            nc.scalar.dma_start(out=relT_s[:], in_=relT[:])
            nc.scalar.dma_start(out=w1_s[:, 4096:10240], in_=w1b[:, 4096:10240])
            w2_s = constp.tile([128, 2816], BF16, tag="w2")
            nc.scalar.dma_start(out=w2_s[:], in_=w2b[:])
            tailT_s = constp.tile([128, 4, NPAD], BF16, tag="tailT")
            nc.scalar.dma_start(out=tailT_s[:], in_=tailT[:])
            core2_h = []
            for h in range(2):
                ct = bigp.tile([128, HALF], BF16, tag="big")
                nc.sync.dma_start(out=ct[:], in_=core2[:, h * HALF:(h + 1) * HALF])
                core2_h.append(ct)

            ident = constp.tile([128, 128], F32)
            make_identity(nc, ident[:])
            ones_row = constp.tile([1, 128], BF16, tag="ones")
            nc.vector.memset(ones_row[:], 1.0)

            def cbc(name, w=None):
                o = _CB[name]
                return cb_s[:, o:o + w] if w else cb_s[:, o:o + 1]

            # persistent tiles
            hsT_s = pers.tile([128, BSH], BF16)        # hs^T (pre-BN), my shard
            tsT_s = pers.tile([128, NPAD], BF16)       # ts^T (+bias)
            tamT_s = pers.tile([128, NPAD], BF16)      # tam^T
            taS = pers.tile([128, NPAD], BF16)         # ta in [n, c] tiles
            WmT_all = pers.tile([128, B], BF16)        # gathered Wm^T raw
            intT_all = pers.tile([128, B], BF16)       # gathered inter^T (score lhsT)
            WmT_nb = pers.tile([128, B], BF16)         # BN1-applied, score lhsT

            # ---------- generic 2-layer MLP producing outT [c, nb] ----------
            def mlp2_T(w1name, b1name, w2name, xt_s, x0, nb, out_ap, b2name):
                w1o, w1nk, w1cols = _W1[w1name]
                w2o, w2m, w2cols = _W2[w2name]
                nm = w1cols // 128
                b1 = _CB[b1name]
                h1_t = []
                for m in range(nm):
                    ps = psp.tile([128, nb], F32, tag="ps")
                    for k in range(w1nk):
                        nc.tensor.matmul(
                            ps[:],
                            w1_s[:, w1o + k * w1cols + m * 128:
                                 w1o + k * w1cols + (m + 1) * 128],
                            xt_s[:, k, x0:x0 + nb],
                            start=(k == 0), stop=(k == w1nk - 1))
                    h1 = h1p.tile([128, nb], BF16, tag="h1")
                    nc.scalar.activation(h1[:], ps[:], AF.Relu,
                                         bias=cb_s[:, b1 + m:b1 + m + 1])
                    h1_t.append(h1)
                ps2 = psp.tile([128, nb], F32, tag="ps")
                for m in range(nm):
                    nc.tensor.matmul(
                        ps2[:], w2_s[:, w2o + m * w2cols:w2o + m * w2cols + w2cols],
                        h1_t[m][:], start=(m == 0), stop=(m == nm - 1))
                if b2name is None:
                    nc.scalar.copy(out_ap, ps2[:])
                else:
                    nc.scalar.add(out_ap, ps2[:], cbc(b2name))
                return h1_t

            # ---------------- head MLP (shard) + BN0 partial stats ----------------
            mlp2_T("hsw1", "hsb1", "hsw2", headT_s, 0, BSH, hsT_s[:], None)

            # partial batch stats over my 256 rows; AllGather 8x[128,6], aggregate.
            # This tiny first collective also absorbs the cross-core launch skew.
            st0 = smallp.tile([128, 6], F32, tag="sm6a")
            nc.vector.bn_stats(st0[:], hsT_s[:])
            ag_s_in = dramp.tile([128, 6], F32)
            ag_s_out = dramp.tile([NCORES, 128, 6], F32, addr_space="Shared")
            nc.sync.dma_start(out=ag_s_in[:], in_=st0[:])
            nc.gpsimd.collective_compute(
                "AllGather", ALU.bypass,
                replica_groups=[list(range(NCORES))],
                ins=[ag_s_in.opt()], outs=[ag_s_out.opt()])
            st8 = smallp.tile([128, NCORES, 6], F32, tag="sm6b")
            nc.sync.dma_start(out=st8[:], in_=ag_s_out[:].rearrange("r d s -> d r s"))

            def scale_shift_from_stats(st_ap, g_ap, b_ap):
                mv = smallp.tile([128, 2], F32, tag="sm2")
                nc.vector.bn_aggr(mv[:], st_ap)
                scale = smallp.tile([128, 1], F32, tag="sm1a")
                shift = smallp.tile([128, 1], F32, tag="sm1b")
                tmp = smallp.tile([128, 1], F32, tag="sm1c")
                nc.vector.tensor_scalar_add(tmp[:], mv[:, 1:2], 1e-5)
                nc.scalar.activation(scale[:], tmp[:], AF.Sqrt)
                nc.vector.reciprocal(scale[:], scale[:])
                nc.vector.tensor_mul(scale[:], scale[:], g_ap)
                nc.vector.tensor_mul(tmp[:], mv[:, 0:1], scale[:])
                nc.vector.tensor_sub(shift[:], b_ap, tmp[:])
                return scale, shift

            def bn_scale_shift(xT_ap, nfree, g_ap, b_ap):
                nchunk = nfree // 512
                st = smallp.tile([128, nchunk, 6], F32, tag="sm6")
                for i in range(nchunk):
                    nc.vector.bn_stats(st[:, i, :], xT_ap[:, i * 512:(i + 1) * 512])
                return scale_shift_from_stats(st[:], g_ap, b_ap)

            # ---------------- rel MLP (shard) -> rsT (bf16) ----------------
            rsT_bf = smallp.tile([128, BSH], BF16, tag="rsTbf")
            mlp2_T("rsw1", "rsb1", "rsw2", relT_s, 0, BSH, rsT_bf[:], "rsb2")

            # ---------------- hr MLP (shard) -> hraT -> hrm tiles ----------------
            w1o, _, w1c = _W1["hrw1"]
            hr_h1 = []
            for m in range(2):
                ps = psp.tile([128, BSH], F32, tag="ps")
                for k in range(8):
                    xt = (headT_s[:, k, 0:BSH] if k < 4
                          else relT_s[:, k - 4, :])
                    nc.tensor.matmul(ps[:],
                                     w1_s[:, w1o + k * w1c + m * 128:
                                          w1o + k * w1c + (m + 1) * 128],
                                     xt, start=(k == 0), stop=(k == 7))
                h1 = h1p.tile([128, BSH], BF16, tag="h1")
                nc.scalar.activation(h1[:], ps[:], AF.Relu,
                                     bias=cb_s[:, _CB["hrb1"] + m:_CB["hrb1"] + m + 1])
                hr_h1.append(h1)
            w2o, _, w2c = _W2["hrw2"]
            hr_h2 = []
            for m in range(2):
                ps = psp.tile([128, BSH], F32, tag="ps")
                for k in range(2):
                    nc.tensor.matmul(ps[:],
                                     w2_s[:, w2o + k * w2c + m * 128:
                                          w2o + k * w2c + (m + 1) * 128],
                                     hr_h1[k][:], start=(k == 0), stop=(k == 1))
                h2 = h1p.tile([128, BSH], BF16, tag="h1")
                nc.scalar.activation(h2[:], ps[:], AF.Relu,
                                     bias=cb_s[:, _CB["hrb2"] + m:_CB["hrb2"] + m + 1])
                hr_h2.append(h2)
            w3o, _, _ = _W2["hrw3"]
            hraT = smallp.tile([128, BSH], F32, tag="hraT")
            ps3 = psp.tile([128, BSH], F32, tag="ps")
            for k in range(2):
                nc.tensor.matmul(ps3[:], w2_s[:, w3o + k * 128:w3o + (k + 1) * 128],
                                 hr_h2[k][:], start=(k == 0), stop=(k == 1))
            nc.scalar.add(hraT[:], ps3[:], cbc("hrb3"))

            # ---------- soft top-10 mask helper ----------
            def topk_mask_mul(src_sb, out_ap):
                """out = sigmoid((src - thr10)/TEMP) * src on [128,128]."""
                m8 = smallp.tile([128, 8], BF16, tag="m8")
                zap = smallp.tile([128, 128], BF16, tag="zap")
                nc.vector.max(out=m8[:], in_=src_sb)
                nc.vector.match_replace(out=zap[:], in_to_replace=m8[:],
                                        in_values=src_sb, imm_value=NEG)
                nc.vector.max(out=m8[:], in_=zap[:])
                thr = smallp.tile([128, 1], F32, tag="thr")
                nc.vector.tensor_scalar_mul(thr[:], m8[:, 1:2], -1.0 / TEMP)
                mask = smallp.tile([128, 128], BF16, tag="mask")
                nc.scalar.activation(mask[:], src_sb, AF.Sigmoid,
                                     bias=thr[:, 0:1], scale=1.0 / TEMP)
                nc.vector.tensor_mul(out_ap, mask[:], src_sb)

            hrm_bf = []
            for t in range(2):
                pst = ptp.tile([128, 128], F32, tag="pt")
                nc.tensor.transpose(pst[:], hraT[:, t * 128:(t + 1) * 128], ident[:])
                hra = smallp.tile([128, 128], BF16, tag="hra")
                nc.scalar.copy(hra[:], pst[:])
                hb = smallp.tile([128, 128], BF16, tag="hrmbf")
                topk_mask_mul(hra[:], hb[:])
                hrm_bf.append(hb)

            # ---------------- Wm (shard): V blocks + stt accumulation ----------------
            # BN0 scale/shift from the gathered stats, then ha tiles
            bn0_scale, bn0_shift = scale_shift_from_stats(st8[:], cbc("bn0g"),
                                                          cbc("bn0b"))
            haT_aff = smallp.tile([128, BSH], F32, tag="haT")
            nc.vector.tensor_scalar(haT_aff[:], hsT_s[:], bn0_scale[:, 0:1],
                                    bn0_shift[:, 0:1], op0=ALU.mult, op1=ALU.add)
            ha_t = []
            for t in range(2):
                pst = ptp.tile([128, 128], F32, tag="pt")
                nc.tensor.transpose(pst[:], haT_aff[:, t * 128:(t + 1) * 128], ident[:])
                ha = smallp.tile([128, 128], F32, tag="ha")
                nc.scalar.copy(ha[:], pst[:])
                ha_t.append(ha)

            acc_bf = []
            for t in range(2):
                acc = smallp.tile([128, 128], F32, tag="wacc")
                for blk in range(C * C // 512):
                    hsel, hblk = divmod(blk, 16)
                    ps = wmp.tile([128, 512], F32, tag="wps")
                    nc.tensor.matmul(ps[:], rsT_bf[:, t * 128:(t + 1) * 128],
                                     core2_h[hsel][:, hblk * 512:(hblk + 1) * 512],
                                     start=True, stop=True)
                    for j in range(4):
                        cidx = blk * 4 + j
                        if cidx == 0:
                            nc.vector.tensor_scalar(
                                acc[:], ps[:, j * 128:(j + 1) * 128],
                                ha_t[t][:, 0:1], None, op0=ALU.mult)
                        else:
                            nc.vector.scalar_tensor_tensor(
                                acc[:], ps[:, j * 128:(j + 1) * 128],
                                ha_t[t][:, cidx:cidx + 1], acc[:],
                                op0=ALU.mult, op1=ALU.add)
                ab = smallp.tile([128, 128], BF16, tag="waccb")
                nc.vector.tensor_copy(ab[:], acc[:])
                acc_bf.append(ab)

            # ------- tail MLPs (shard, 5 groups of 512), masks deferred -------
            taw1o, _, taw1c = _W1["taw1"]
            taw2o, _, _ = _W2["taw2"]
            for g in range(NG):
                mlp2_T("tsw1", "tsb1", "tsw2", tailT_s, g * 512, 512,
                       tsT_s[:, g * 512:(g + 1) * 512], "tsb2")
                h1_ta = []
                for m in range(4):
                    ps = psp.tile([128, 512], F32, tag="ps")
                    for k in range(4):
                        nc.tensor.matmul(
                            ps[:],
                            w1_s[:, taw1o + k * taw1c + m * 128:
                                 taw1o + k * taw1c + (m + 1) * 128],
                            tailT_s[:, k, g * 512:(g + 1) * 512],
                            start=(k == 0), stop=(k == 3))
                    h1 = h1p.tile([128, 512], BF16, tag="h1")
                    nc.scalar.activation(h1[:], ps[:], AF.Relu,
                                         bias=cb_s[:, _CB["tab1"] + m:
                                                   _CB["tab1"] + m + 1])
                    h1_ta.append(h1)
                for j in range(4):
                    ps_ta = ptp.tile([128, 128], F32, tag="pt")
                    for m in range(4):
                        nc.tensor.matmul(ps_ta[:],
                                         h1_ta[m][:, j * 128:(j + 1) * 128],
                                         w2_s[:, taw2o + m * 128:taw2o + (m + 1) * 128],
                                         start=(m == 0), stop=False)
                    nc.tensor.matmul(ps_ta[:], ones_row[:], tab2_s[:],
                                     start=False, stop=True)
                    nc.scalar.copy(
                        taS[:, g * 512 + j * 128:g * 512 + (j + 1) * 128], ps_ta[:])

            # ---------------- inter: pc load + fused multiply-reduce ----------------
            # last (t,h) unit runs on gpsimd to shorten the DVE serial chain
            intr0 = smallp.tile([128, 128], F32, tag="intr0")
            intr1 = smallp.tile([128, 128], F32, tag="intr1")
            intr_t = [intr0, intr1]
            iscr = smallp.tile([128, 128], BF16, tag="iscr")
            iscr2 = smallp.tile([128, 128], BF16, tag="iscr2")
            intr_bf = []
            for t in range(2):
                for h in range(2):
                    pc = bigp.tile([128, HALF], BF16, tag="big")
                    nc.sync.dma_start(out=pc[:], in_=pcg[t, h])
                    pc3 = pc[:].rearrange("p (d c) -> p d c", c=128)
                    for d in range(64):
                        nc.vector.scalar_tensor_tensor(
                            out=iscr[:], in0=pc3[:, d, :], scalar=1.0,
                            in1=hrm_bf[t][:], op0=ALU.mult, op1=ALU.mult,
                            accum_out=intr_t[t][:, h * 64 + d:h * 64 + d + 1])
                ib = smallp.tile([128, 128], BF16, tag="intrb")
                nc.vector.tensor_copy(ib[:], intr_t[t][:])
                intr_bf.append(ib)

            # ---------------- AllGathers (DMA-xbar transposes, off the PE queue) ----
            WmT_sh = smallp.tile([128, BSH], BF16, tag="WmTsh")
            for t in range(2):
                nc.sync.dma_start_transpose(WmT_sh[:, t * 128:(t + 1) * 128],
                                            acc_bf[t][:])
            ag_w_in = dramp.tile([128, BSH], BF16)
            ag_w_out = dramp.tile([NCORES, 128, BSH], BF16, addr_space="Shared")
            nc.sync.dma_start(out=ag_w_in[:], in_=WmT_sh[:])
            nc.gpsimd.collective_compute(
                "AllGather", ALU.bypass,
                replica_groups=[list(range(NCORES))],
                ins=[ag_w_in.opt()], outs=[ag_w_out.opt()])
            nc.sync.dma_start(out=WmT_all[:],
                              in_=ag_w_out[:].rearrange("r d b -> d r b"))

            # inter AG chain rides the ACT DGE ring so it cannot block the sync ring
            intT_sh = smallp.tile([128, BSH], BF16, tag="intTsh")
            for t in range(2):
                nc.scalar.dma_start_transpose(intT_sh[:, t * 128:(t + 1) * 128],
                                              intr_bf[t][:])
            ag_i_in = dramp.tile([128, BSH], BF16)
            ag_i_out = dramp.tile([NCORES, 128, BSH], BF16, addr_space="Shared")
            nc.scalar.dma_start(out=ag_i_in[:], in_=intT_sh[:])
            nc.gpsimd.collective_compute(
                "AllGather", ALU.bypass,
                replica_groups=[list(range(NCORES))],
                ins=[ag_i_in.opt()], outs=[ag_i_out.opt()])
            nc.scalar.dma_start(out=intT_all[:],
                                in_=ag_i_out[:].rearrange("r d b -> d r b"))

            # BN1 on gathered WmT (full B)
            bn1_scale, bn1_shift = bn_scale_shift(WmT_all[:], B, cbc("bn1g"),
                                                  cbc("bn1b"))
            nc.vector.tensor_scalar(WmT_nb[:], WmT_all[:], bn1_scale[:, 0:1],
                                    bn1_shift[:, 0:1], op0=ALU.mult, op1=ALU.add)

            # ---------------- deferred topk masks (tamT, poss score rhs) ------------
            for i in range(NPAD // 128):
                tam_nt = smallp.tile([128, 128], F32, tag="tamnt")
                topk_mask_mul(taS[:, i * 128:(i + 1) * 128], tam_nt[:])
                pst2 = ptp.tile([128, 128], F32, tag="pt")
                nc.tensor.transpose(pst2[:], tam_nt[:], ident[:])
                nc.scalar.copy(tamT_s[:, i * 128:(i + 1) * 128], pst2[:])

            # ---------------- scores: tucker then poss, staged DMA out --------------
            def score_pass(lhsT_all, rhsT, out_dram):
                for bt in range(NB_FULL):
                    st = stagep.tile([128, NPAD], BF16, tag="sst")
                    for g in range(NG):
                        ps_t = psp.tile([128, 512], F32, tag="ps")
                        nc.tensor.matmul(ps_t[:], lhsT_all[:, bt * 128:(bt + 1) * 128],
                                         rhsT[:, g * 512:(g + 1) * 512],
                                         start=True, stop=True)
                        if g % 2 == 0:
                            nc.scalar.copy(st[:, g * 512:(g + 1) * 512], ps_t[:])
                        else:
                            nc.vector.tensor_copy(st[:, g * 512:(g + 1) * 512],
                                                  ps_t[:])
                    nc.sync.dma_start(
                        out=out_dram[bt * 128:(bt + 1) * 128, :],
                        in_=st[:, 0:NSH])

            score_pass(WmT_nb, tsT_s, tucker)
            score_pass(intT_all, tamT_s, poss)

    nc.finalize()
    return nc


# ---------------------------------------------------------------------------
# host side
# ---------------------------------------------------------------------------

def _to_np(x, dt=np.float32):
    return np.ascontiguousarray(np.asarray(x), dtype=dt)


def prepare_in_maps(inputs):
    bf = np.dtype(ml_dtypes.bfloat16)
    head = _to_np(inputs["head_vector"])        # [B, E]
    rel = _to_np(inputs["relation_vector"])     # [B, E]
    ridx = np.ascontiguousarray(np.asarray(inputs["relation_index"]).astype(np.int64))
    tailv = _to_np(inputs["tail_vector"])       # [N, E]
    codebook = _to_np(inputs["codebook"])       # [R2, C, C]
    core = _to_np(inputs["core"])               # [C, C, C]

    cbt2 = np.tanh(codebook).transpose(0, 2, 1).reshape(R2, 2, HALF).astype(bf)
    core2_host = np.ascontiguousarray(core.reshape(C, C * C)).astype(bf)

    def kchunks(xT):                            # [E, cols] -> [128, nk, cols]
        return np.ascontiguousarray(
            xT.reshape(xT.shape[0] // 128, 128, xT.shape[1]).transpose(1, 0, 2))

    headT_full = head.T.astype(bf)              # [E, B]
    relT_full = rel.T.astype(bf)                # [E, B]
    tailT_full = tailv.T.astype(bf)             # [E, N]

    # w1 blob [128, 10240]
    w1b = np.zeros((128, 10240), bf)
    for name, (off, nk, cols) in _W1.items():
        w = _to_np(inputs[name]).astype(bf)     # [nk*128, cols]
        for k in range(nk):
            w1b[:, off + k * cols:off + (k + 1) * cols] = w[k * 128:(k + 1) * 128]
    # w2 blob [128, 2816]: [E, C] viewed as [128, m, cols] flattened
    w2b = np.zeros((128, 2816), bf)
    for name, (off, m, cols) in _W2.items():
        w = _to_np(inputs[name]).astype(bf)     # [m*128, cols]
        v = w.reshape(m, 128, cols).transpose(1, 0, 2).reshape(128, m * cols)
        w2b[:, off:off + m * cols] = v
    # const blob [128, 27] f32
    cbb = np.zeros((128, 27), np.float32)

    def put_chunked(name, src, nk):
        cbb[:, _CB[name]:_CB[name] + nk] = _to_np(src).reshape(nk, 128).T

    put_chunked("hsb1", inputs["hsb1"], 4)
    put_chunked("rsb1", inputs["rsb1"], 4)
    put_chunked("tsb1", inputs["tsb1"], 4)
    put_chunked("tab1", inputs["tab1"], 4)
    put_chunked("hrb1", inputs["hrb1"], 2)
    put_chunked("hrb2", inputs["hrb2"], 2)
    for name, key in [("hrb3", "hrb3"), ("rsb2", "rsb2"), ("tsb2", "tsb2"),
                      ("bn0g", "bn0_g"), ("bn0b", "bn0_b"),
                      ("bn1g", "bn1_g"), ("bn1b", "bn1_b")]:
        cbb[:, _CB[name]] = _to_np(inputs[key])

    weights_common = {
        "w1b": w1b, "w2b": w2b, "cb": cbb,
        "tab2r": _to_np(inputs["tab2"]).reshape(1, 128).astype(bf),
        "core2": core2_host,
    }

    in_maps = []
    for k in range(NCORES):
        b0 = k * BSH
        n0 = k * NSH
        headT_k = kchunks(np.ascontiguousarray(headT_full[:, b0:b0 + BSH]))
        tailT_k = np.zeros((E, NPAD), bf)
        tailT_k[:, :NSH] = tailT_full[:, n0:n0 + NSH]
        pcg_k = np.ascontiguousarray(
            cbt2[ridx[b0:b0 + BSH]].reshape(2, 128, 2, HALF).transpose(0, 2, 1, 3))
        m = dict(weights_common)
        m["headT"] = headT_k
        m["relT"] = kchunks(np.ascontiguousarray(relT_full[:, b0:b0 + BSH]))
        m["tailT"] = kchunks(tailT_k)
        m["pcg"] = pcg_k
        in_maps.append(m)
    return in_maps


def assemble_outputs(results):
    tuckers, posses = [], []
    for k in range(NCORES):
        r = results[k]
        tuckers.append(np.asarray(r["tucker"]).astype(np.float32))
        posses.append(np.asarray(r["poss"]).astype(np.float32))
    tucker_full = np.concatenate(tuckers, axis=1)
    poss_full = np.concatenate(posses, axis=1)
    return tucker_full, poss_full


def kernel(**inputs):
    if "prog" not in _PROG_CACHE:
        _PROG_CACHE["prog"] = build_program()
    nc = _PROG_CACHE["prog"]
    in_maps = prepare_in_maps(inputs)
    res = run_bass_kernel_spmd(nc, in_maps, list(range(NCORES)))
    return assemble_outputs(res.results)
